# revision 1
# baseline (speedup 1.0000x reference)
"""GroupedQueryAttention kernel for 8 Trainium2 NeuronCores.

Sharding: core c = (batch b = c//2, seq-half sh = c%2). Each core computes the
full attention output for 1024 query rows of one batch: all 8 q heads
(2 kv heads), plus the q/k/v projections and the o-projection for those rows.
Host work is limited to slicing/transposing inputs and concatenating outputs.

On-device layout: scoresT [keys, queries] so softmax-exp'd probabilities feed
attn@v matmuls directly as the moving operand (no transposes anywhere).
Denominators are produced by ones-weight matmuls replicated across all 64
output partitions, so normalization is a plain elementwise multiply.
"""

import numpy as np

B, S, D = 4, 2048, 512
H, KV, DH = 8, 2, 64
SQ = S // 2  # queries per core
NCORES = 8
PAIRS = 4  # head pairs (p, p+4); p -> kv0 rows 0:64, p+4 -> kv1 rows 64:128
SCALE = 1.0 / 8.0  # 1/sqrt(DH)
PERM = [0, 4, 1, 5, 2, 6, 3, 7]  # head order: pair-major

_built = {}


def _build_nc():
    import concourse.mybir as mybir
    import concourse.tile as tile
    from concourse import bacc

    fp32 = mybir.dt.float32
    Exp = mybir.ActivationFunctionType.Exp

    nc = bacc.Bacc("TRN2", target_bir_lowering=False, debug=False,
                   num_devices=NCORES)

    xT = nc.dram_tensor("xT", [D, S], fp32, kind="ExternalInput").ap()
    wq = nc.dram_tensor("wq", [D, D], fp32, kind="ExternalInput").ap()
    wk = nc.dram_tensor("wk", [D, KV * DH], fp32, kind="ExternalInput").ap()
    wv = nc.dram_tensor("wv", [D, KV * DH], fp32, kind="ExternalInput").ap()
    wo = nc.dram_tensor("wo", [D, D], fp32, kind="ExternalInput").ap()
    bqp = nc.dram_tensor("bqp", [128, PAIRS], fp32, kind="ExternalInput").ap()
    bkvp = nc.dram_tensor("bkvp", [128, 1], fp32, kind="ExternalInput").ap()
    bvbc = nc.dram_tensor("bvbc", [128, 128], fp32, kind="ExternalInput").ap()
    bobc = nc.dram_tensor("bobc", [128, D], fp32, kind="ExternalInput").ap()
    y = nc.dram_tensor("y", [SQ, D], fp32, kind="ExternalOutput").ap()

    with tile.TileContext(nc) as tc:
        with (
            tc.tile_pool(name="consts", bufs=1) as consts,
            tc.tile_pool(name="epool", bufs=3) as epool,
            tc.tile_pool(name="opool", bufs=6) as opool,
            tc.tile_pool(name="rpool", bufs=2) as rpool,
            tc.tile_pool(name="ypool", bufs=3) as ypool,
            tc.tile_pool(name="pssc", bufs=2, space="PSUM") as pssc,
            tc.tile_pool(name="ps512", bufs=4, space="PSUM") as ps512,
        ):
            # ---- load constants / inputs ----
            xt_sb = consts.tile([128, 4, S], fp32, tag="xt")
            nc.sync.dma_start(xt_sb[:], xT.rearrange("(c p) s -> p c s", p=128))
            wq_sb = consts.tile([128, 4, D], fp32, tag="wq")
            nc.sync.dma_start(wq_sb[:], wq.rearrange("(c p) j -> p c j", p=128))
            wk_sb = consts.tile([128, 4, 128], fp32, tag="wk")
            nc.sync.dma_start(wk_sb[:], wk.rearrange("(c p) j -> p c j", p=128))
            wv_sb = consts.tile([128, 4, 128], fp32, tag="wv")
            nc.sync.dma_start(wv_sb[:], wv.rearrange("(c p) j -> p c j", p=128))
            wo_sb = consts.tile([128, 4, D], fp32, tag="wo")
            nc.sync.dma_start(wo_sb[:], wo.rearrange("(c p) j -> p c j", p=128))
            bq_sb = consts.tile([128, PAIRS], fp32, tag="bq")
            nc.sync.dma_start(bq_sb[:], bqp)
            bkv_sb = consts.tile([128, 1], fp32, tag="bkv")
            nc.sync.dma_start(bkv_sb[:], bkvp)
            bv_sb = consts.tile([128, 128], fp32, tag="bv")
            nc.sync.dma_start(bv_sb[:], bvbc)
            bo_sb = consts.tile([128, D], fp32, tag="bo")
            nc.sync.dma_start(bo_sb[:], bobc)
            ones_sb = consts.tile([128, DH], fp32, tag="ones")
            nc.vector.memset(ones_sb[:], 1.0)

            # ---- projections ----
            # kT [128 (kv0|kv1 head-dim), S]
            kt_sb = consts.tile([128, S], fp32, tag="kt")
            for sc in range(S // 512):
                ps = ps512.tile([128, 512], fp32, tag="ps512")
                for c in range(4):
                    nc.tensor.matmul(ps[:], wk_sb[:, c, :],
                                     xt_sb[:, c, sc * 512:(sc + 1) * 512],
                                     start=(c == 0), stop=(c == 3))
                nc.vector.tensor_scalar_add(kt_sb[:, sc * 512:(sc + 1) * 512],
                                            ps[:], bkv_sb[:, 0:1])
            # v natural [s-block, 128][(kv0|kv1) head-dim]
            v_sb = consts.tile([128, S // 128, 128], fp32, tag="v")
            for sb in range(S // 128):
                ps = ps512.tile([128, 512], fp32, tag="ps512")
                for c in range(4):
                    nc.tensor.matmul(ps[:, 0:128],
                                     xt_sb[:, c, sb * 128:(sb + 1) * 128],
                                     wv_sb[:, c, :],
                                     start=(c == 0), stop=(c == 3))
                nc.vector.tensor_add(v_sb[:, sb, :], ps[:, 0:128], bv_sb[:])
            # qT [128 (head p | head p+4), SQ] per pair chunk
            qt_sb = consts.tile([128, PAIRS, SQ], fp32, tag="qt")
            for pr in range(PAIRS):
                for sc in range(SQ // 512):
                    ps = ps512.tile([128, 512], fp32, tag="ps512")
                    for c in range(4):
                        nc.tensor.matmul(ps[:],
                                         wq_sb[:, c, pr * 128:(pr + 1) * 128],
                                         xt_sb[:, c, sc * 512:(sc + 1) * 512],
                                         start=(c == 0), stop=(c == 3))
                    nc.vector.tensor_scalar_add(
                        qt_sb[:, pr, sc * 512:(sc + 1) * 512], ps[:],
                        bq_sb[:, pr:pr + 1])

            # ---- attention + o-proj ----
            NKB = S // 128  # 16 key blocks
            for qc in range(SQ // 512):
                ot_tiles = []
                for pr in range(PAIRS):
                    acc = ps512.tile([128, 512], fp32, tag="ps512")
                    den = ps512.tile([128, 512], fp32, tag="ps512")
                    e_tiles = [None] * NKB

                    def attnv(kb):
                        e = e_tiles[kb]
                        nc.tensor.matmul(acc[0:64, :], v_sb[:, kb, 0:64],
                                         e[:, 0:512],
                                         start=(kb == 0), stop=(kb == NKB - 1),
                                         tile_position=(0, 0))
                        nc.tensor.matmul(acc[64:128, :], v_sb[:, kb, 64:128],
                                         e[:, 512:1024],
                                         start=(kb == 0), stop=(kb == NKB - 1),
                                         tile_position=(0, 64))
                        nc.tensor.matmul(den[0:64, :], ones_sb[:],
                                         e[:, 0:512],
                                         start=(kb == 0), stop=(kb == NKB - 1),
                                         tile_position=(0, 0))
                        nc.tensor.matmul(den[64:128, :], ones_sb[:],
                                         e[:, 512:1024],
                                         start=(kb == 0), stop=(kb == NKB - 1),
                                         tile_position=(0, 64))

                    for kb in range(NKB):
                        sc_ps = pssc.tile([128, 1024], fp32, tag="scores")
                        nc.tensor.matmul(
                            sc_ps[:, 0:512],
                            kt_sb[0:64, kb * 128:(kb + 1) * 128],
                            qt_sb[0:64, pr, qc * 512:(qc + 1) * 512])
                        nc.tensor.matmul(
                            sc_ps[:, 512:1024],
                            kt_sb[64:128, kb * 128:(kb + 1) * 128],
                            qt_sb[64:128, pr, qc * 512:(qc + 1) * 512])
                        e = epool.tile([128, 1024], fp32, tag="E")
                        e_tiles[kb] = e
                        nc.scalar.activation(e[:], sc_ps[:], Exp, scale=SCALE)
                        # software pipeline: consume previous block's probs so
                        # PE never waits on the exp of the current block
                        if kb >= 1:
                            attnv(kb - 1)
                    attnv(NKB - 1)

                    rb = rpool.tile([128, 512], fp32, tag="recip")
                    scr = rpool.tile([128, 512], fp32, tag="rscr")
                    nc.vector.reciprocal_approx_accurate(rb[:], den[:], scr[:])
                    ot = opool.tile([128, 512], fp32, tag="outT")
                    nc.vector.tensor_mul(ot[:], acc[:], rb[:])
                    ot_tiles.append(ot)
                for m in range(4):
                    yp = ps512.tile([128, 512], fp32, tag="ps512")
                    for pr2 in range(PAIRS):
                        nc.tensor.matmul(yp[:],
                                         ot_tiles[pr2][:, m * 128:(m + 1) * 128],
                                         wo_sb[:, pr2, :],
                                         start=(pr2 == 0), stop=(pr2 == 3))
                    yt = ypool.tile([128, 512], fp32, tag="y")
                    nc.vector.tensor_add(yt[:], yp[:], bo_sb[:])
                    blk = qc * 4 + m
                    nc.sync.dma_start(y[blk * 128:(blk + 1) * 128, :], yt[:])

    nc.finalize()
    return nc


def _get_nc():
    if "nc" not in _built:
        _built["nc"] = _build_nc()
    return _built["nc"]


def kernel(x, Wq, bq, Wk, bk, Wv, bv, Wo, bo):
    from concourse.bass_utils import run_bass_kernel_spmd

    x = np.ascontiguousarray(np.asarray(x, np.float32))
    Wq = np.asarray(Wq, np.float32)
    bq = np.asarray(bq, np.float32)
    Wk = np.asarray(Wk, np.float32)
    bk = np.asarray(bk, np.float32)
    Wv = np.asarray(Wv, np.float32)
    bv = np.asarray(bv, np.float32)
    Wo = np.asarray(Wo, np.float32)
    bo = np.asarray(bo, np.float32)

    wq_p = np.ascontiguousarray(
        Wq.reshape(D, H, DH)[:, PERM, :].reshape(D, D))
    wo_p = np.ascontiguousarray(Wo.reshape(H, DH, D)[PERM].reshape(D, D))
    bq_p = np.ascontiguousarray(
        bq.reshape(H, DH)[PERM].reshape(PAIRS, 128).T)
    bkv_p = np.ascontiguousarray(bk.reshape(128, 1))
    bv_bc = np.ascontiguousarray(np.tile(bv[None, :], (128, 1)))
    bo_bc = np.ascontiguousarray(np.tile(bo[None, :], (128, 1)))

    in_maps = []
    for c in range(NCORES):
        b, sh = divmod(c, 2)
        xroll = np.roll(x[b], -sh * SQ, axis=0)
        in_maps.append({
            "xT": np.ascontiguousarray(xroll.T),
            "wq": wq_p, "wk": Wk, "wv": Wv, "wo": wo_p,
            "bqp": bq_p, "bkvp": bkv_p, "bvbc": bv_bc, "bobc": bo_bc,
        })

    nc = _get_nc()
    res = run_bass_kernel_spmd(nc, in_maps, list(range(NCORES)))
    out = np.empty((B, S, D), np.float32)
    for c in range(NCORES):
        b, sh = divmod(c, 2)
        out[b, sh * SQ:(sh + 1) * SQ, :] = res.results[c]["y"]
    return out



# revision 2
# speedup vs baseline: 2.5491x; 2.5491x over previous
"""GroupedQueryAttention kernel for 8 Trainium2 NeuronCores.

Sharding: core c = (batch b = c//2, seq-half sh = c%2). Each core computes the
full attention output for 1024 query rows of one batch: all 8 q heads
(2 kv heads), plus the q/k/v projections and the o-projection for those rows.
Host work is limited to slicing/transposing/casting inputs and concatenating
outputs.

All matmul operands are bf16 (fp32 matmuls run as two PE passes on TRN2 —
bf16 halves tensor-engine time); accumulation stays fp32 in PSUM and the
softmax denominator/reciprocal stays fp32.

On-device layout: scoresT [keys, queries] so softmax-exp'd probabilities feed
attn@v matmuls directly as the moving operand (no transposes anywhere).
Denominators are produced by ones-weight matmuls replicated across all 64
output partitions, so normalization is a plain elementwise multiply.
"""

import numpy as np

B, S, D = 4, 2048, 512
H, KV, DH = 8, 2, 64
SQ = S // 2  # queries per core
NCORES = 8
PAIRS = 4  # head pairs (p, p+4); p -> kv0 rows 0:64, p+4 -> kv1 rows 64:128
SCALE = 1.0 / 8.0  # 1/sqrt(DH)
PERM = [0, 4, 1, 5, 2, 6, 3, 7]  # head order: pair-major

_built = {}


def _build_nc():
    import concourse.mybir as mybir
    import concourse.tile as tile
    from concourse import bacc

    fp32 = mybir.dt.float32
    bf16 = mybir.dt.bfloat16
    Exp = mybir.ActivationFunctionType.Exp

    nc = bacc.Bacc("TRN2", target_bir_lowering=False, debug=False,
                   num_devices=NCORES)

    xT = nc.dram_tensor("xT", [D, S], bf16, kind="ExternalInput").ap()
    wq = nc.dram_tensor("wq", [D, D], bf16, kind="ExternalInput").ap()
    wk = nc.dram_tensor("wk", [D, KV * DH], bf16, kind="ExternalInput").ap()
    wv = nc.dram_tensor("wv", [D, KV * DH], bf16, kind="ExternalInput").ap()
    wo = nc.dram_tensor("wo", [D, D], bf16, kind="ExternalInput").ap()
    bqp = nc.dram_tensor("bqp", [128, PAIRS], fp32, kind="ExternalInput").ap()
    bkvp = nc.dram_tensor("bkvp", [128, 1], fp32, kind="ExternalInput").ap()
    bvbc = nc.dram_tensor("bvbc", [128, 128], fp32, kind="ExternalInput").ap()
    bobc = nc.dram_tensor("bobc", [128, D], fp32, kind="ExternalInput").ap()
    y = nc.dram_tensor("y", [SQ, D], fp32, kind="ExternalOutput").ap()

    with tile.TileContext(nc) as tc:
        with (
            tc.tile_pool(name="consts", bufs=1) as consts,
            tc.tile_pool(name="epool", bufs=3) as epool,
            tc.tile_pool(name="opool", bufs=6) as opool,
            tc.tile_pool(name="rpool", bufs=2) as rpool,
            tc.tile_pool(name="ypool", bufs=3) as ypool,
            tc.tile_pool(name="pssc", bufs=2, space="PSUM") as pssc,
            tc.tile_pool(name="ps512", bufs=4, space="PSUM") as ps512,
        ):
            # ---- load constants / inputs ----
            xt_sb = consts.tile([128, 4, S], bf16, tag="xt")
            nc.sync.dma_start(xt_sb[:], xT.rearrange("(c p) s -> p c s", p=128))
            wq_sb = consts.tile([128, 4, D], bf16, tag="wq")
            nc.sync.dma_start(wq_sb[:], wq.rearrange("(c p) j -> p c j", p=128))
            wk_sb = consts.tile([128, 4, 128], bf16, tag="wk")
            nc.sync.dma_start(wk_sb[:], wk.rearrange("(c p) j -> p c j", p=128))
            wv_sb = consts.tile([128, 4, 128], bf16, tag="wv")
            nc.sync.dma_start(wv_sb[:], wv.rearrange("(c p) j -> p c j", p=128))
            wo_sb = consts.tile([128, 4, D], bf16, tag="wo")
            nc.sync.dma_start(wo_sb[:], wo.rearrange("(c p) j -> p c j", p=128))
            bq_sb = consts.tile([128, PAIRS], fp32, tag="bq")
            nc.sync.dma_start(bq_sb[:], bqp)
            bkv_sb = consts.tile([128, 1], fp32, tag="bkv")
            nc.sync.dma_start(bkv_sb[:], bkvp)
            bv_sb = consts.tile([128, 128], fp32, tag="bv")
            nc.sync.dma_start(bv_sb[:], bvbc)
            bo_sb = consts.tile([128, D], fp32, tag="bo")
            nc.sync.dma_start(bo_sb[:], bobc)
            ones_sb = consts.tile([128, DH], bf16, tag="ones")
            nc.vector.memset(ones_sb[:], 1.0)

            # ---- projections ----
            # kT [128 (kv0|kv1 head-dim), S]
            kt_sb = consts.tile([128, S], bf16, tag="kt")
            for sc in range(S // 512):
                ps = ps512.tile([128, 512], fp32, tag="ps512")
                for c in range(4):
                    nc.tensor.matmul(ps[:], wk_sb[:, c, :],
                                     xt_sb[:, c, sc * 512:(sc + 1) * 512],
                                     start=(c == 0), stop=(c == 3))
                nc.vector.tensor_scalar_add(kt_sb[:, sc * 512:(sc + 1) * 512],
                                            ps[:], bkv_sb[:, 0:1])
            # v natural [s-block, 128][(kv0|kv1) head-dim]
            v_sb = consts.tile([128, S // 128, 128], bf16, tag="v")
            for sb in range(S // 128):
                ps = ps512.tile([128, 512], fp32, tag="ps512")
                for c in range(4):
                    nc.tensor.matmul(ps[:, 0:128],
                                     xt_sb[:, c, sb * 128:(sb + 1) * 128],
                                     wv_sb[:, c, :],
                                     start=(c == 0), stop=(c == 3))
                nc.vector.tensor_add(v_sb[:, sb, :], ps[:, 0:128], bv_sb[:])
            # qT [128 (head p | head p+4), SQ] per pair chunk
            qt_sb = consts.tile([128, PAIRS, SQ], bf16, tag="qt")
            for pr in range(PAIRS):
                for sc in range(SQ // 512):
                    ps = ps512.tile([128, 512], fp32, tag="ps512")
                    for c in range(4):
                        nc.tensor.matmul(ps[:],
                                         wq_sb[:, c, pr * 128:(pr + 1) * 128],
                                         xt_sb[:, c, sc * 512:(sc + 1) * 512],
                                         start=(c == 0), stop=(c == 3))
                    nc.vector.tensor_scalar_add(
                        qt_sb[:, pr, sc * 512:(sc + 1) * 512], ps[:],
                        bq_sb[:, pr:pr + 1])

            # ---- attention + o-proj ----
            NKB = S // 128  # 16 key blocks
            for qc in range(SQ // 512):
                ot_tiles = []
                for pr in range(PAIRS):
                    acc = ps512.tile([128, 512], fp32, tag="ps512")
                    den = ps512.tile([128, 512], fp32, tag="ps512")
                    e_tiles = [None] * NKB

                    def attnv(kb):
                        e = e_tiles[kb]
                        nc.tensor.matmul(acc[0:64, :], v_sb[:, kb, 0:64],
                                         e[:, 0:512],
                                         start=(kb == 0), stop=(kb == NKB - 1),
                                         tile_position=(0, 0))
                        nc.tensor.matmul(acc[64:128, :], v_sb[:, kb, 64:128],
                                         e[:, 512:1024],
                                         start=(kb == 0), stop=(kb == NKB - 1),
                                         tile_position=(0, 64))
                        nc.tensor.matmul(den[0:64, :], ones_sb[:],
                                         e[:, 0:512],
                                         start=(kb == 0), stop=(kb == NKB - 1),
                                         tile_position=(0, 0))
                        nc.tensor.matmul(den[64:128, :], ones_sb[:],
                                         e[:, 512:1024],
                                         start=(kb == 0), stop=(kb == NKB - 1),
                                         tile_position=(0, 64))

                    for kb in range(NKB):
                        sc_ps = pssc.tile([128, 1024], fp32, tag="scores")
                        nc.tensor.matmul(
                            sc_ps[:, 0:512],
                            kt_sb[0:64, kb * 128:(kb + 1) * 128],
                            qt_sb[0:64, pr, qc * 512:(qc + 1) * 512])
                        nc.tensor.matmul(
                            sc_ps[:, 512:1024],
                            kt_sb[64:128, kb * 128:(kb + 1) * 128],
                            qt_sb[64:128, pr, qc * 512:(qc + 1) * 512])
                        e = epool.tile([128, 1024], bf16, tag="E")
                        e_tiles[kb] = e
                        nc.scalar.activation(e[:], sc_ps[:], Exp, scale=SCALE)
                        # software pipeline: consume previous block's probs so
                        # PE never waits on the exp of the current block
                        if kb >= 1:
                            attnv(kb - 1)
                    attnv(NKB - 1)

                    rb = rpool.tile([128, 512], fp32, tag="recip")
                    scr = rpool.tile([128, 512], fp32, tag="rscr")
                    nc.vector.reciprocal_approx_accurate(rb[:], den[:], scr[:])
                    ot = opool.tile([128, 512], bf16, tag="outT")
                    nc.vector.tensor_mul(ot[:], acc[:], rb[:])
                    ot_tiles.append(ot)
                for m in range(4):
                    yp = ps512.tile([128, 512], fp32, tag="ps512")
                    for pr2 in range(PAIRS):
                        nc.tensor.matmul(yp[:],
                                         ot_tiles[pr2][:, m * 128:(m + 1) * 128],
                                         wo_sb[:, pr2, :],
                                         start=(pr2 == 0), stop=(pr2 == 3))
                    yt = ypool.tile([128, 512], fp32, tag="y")
                    nc.vector.tensor_add(yt[:], yp[:], bo_sb[:])
                    blk = qc * 4 + m
                    nc.sync.dma_start(y[blk * 128:(blk + 1) * 128, :], yt[:])

    nc.finalize()
    return nc


def _get_nc():
    if "nc" not in _built:
        _built["nc"] = _build_nc()
    return _built["nc"]


def _prep_in_maps(x, Wq, bq, Wk, bk, Wv, bv, Wo, bo):
    import ml_dtypes

    bf16 = ml_dtypes.bfloat16

    x = np.ascontiguousarray(np.asarray(x, np.float32))
    Wq = np.asarray(Wq, np.float32)
    bq = np.asarray(bq, np.float32)
    Wk = np.asarray(Wk, np.float32)
    bk = np.asarray(bk, np.float32)
    Wv = np.asarray(Wv, np.float32)
    bv = np.asarray(bv, np.float32)
    Wo = np.asarray(Wo, np.float32)
    bo = np.asarray(bo, np.float32)

    wq_p = np.ascontiguousarray(
        Wq.reshape(D, H, DH)[:, PERM, :].reshape(D, D).astype(bf16))
    wo_p = np.ascontiguousarray(
        Wo.reshape(H, DH, D)[PERM].reshape(D, D).astype(bf16))
    wk_b = np.ascontiguousarray(Wk.astype(bf16))
    wv_b = np.ascontiguousarray(Wv.astype(bf16))
    bq_p = np.ascontiguousarray(
        bq.reshape(H, DH)[PERM].reshape(PAIRS, 128).T)
    bkv_p = np.ascontiguousarray(bk.reshape(128, 1))
    bv_bc = np.ascontiguousarray(np.tile(bv[None, :], (128, 1)))
    bo_bc = np.ascontiguousarray(np.tile(bo[None, :], (128, 1)))

    in_maps = []
    for c in range(NCORES):
        b, sh = divmod(c, 2)
        xroll = np.roll(x[b], -sh * SQ, axis=0)
        in_maps.append({
            "xT": np.ascontiguousarray(xroll.T.astype(bf16)),
            "wq": wq_p, "wk": wk_b, "wv": wv_b, "wo": wo_p,
            "bqp": bq_p, "bkvp": bkv_p, "bvbc": bv_bc, "bobc": bo_bc,
        })
    return in_maps


def kernel(x, Wq, bq, Wk, bk, Wv, bv, Wo, bo):
    from concourse.bass_utils import run_bass_kernel_spmd

    in_maps = _prep_in_maps(x, Wq, bq, Wk, bk, Wv, bv, Wo, bo)
    nc = _get_nc()
    res = run_bass_kernel_spmd(nc, in_maps, list(range(NCORES)))
    out = np.empty((B, S, D), np.float32)
    for c in range(NCORES):
        b, sh = divmod(c, 2)
        out[b, sh * SQ:(sh + 1) * SQ, :] = res.results[c]["y"]
    return out


# revision 3
# speedup vs baseline: 2.5590x; 1.0039x over previous
"""GroupedQueryAttention kernel for 8 Trainium2 NeuronCores.

Sharding: core c = (batch b = c//2, seq-half sh = c%2). Each core computes the
full attention output for 1024 query rows of one batch: all 8 q heads
(2 kv heads), plus the q/k/v projections and the o-projection for those rows.
Host work is limited to slicing/transposing/casting inputs and concatenating
outputs.

All matmul operands are bf16 (fp32 matmuls run as two PE passes on TRN2 —
bf16 halves tensor-engine time); accumulation stays fp32 in PSUM and the
softmax denominator/reciprocal stays fp32.

On-device layout: scoresT [keys, queries] so softmax-exp'd probabilities feed
attn@v matmuls directly as the moving operand (no transposes anywhere).
Denominators are produced by ones-weight matmuls replicated across all 64
output partitions, so normalization is a plain elementwise multiply.
"""

import numpy as np

B, S, D = 4, 2048, 512
H, KV, DH = 8, 2, 64
SQ = S // 2  # queries per core
NCORES = 8
PAIRS = 4  # head pairs (p, p+4); p -> kv0 rows 0:64, p+4 -> kv1 rows 64:128
SCALE = 1.0 / 8.0  # 1/sqrt(DH)
PERM = [0, 4, 1, 5, 2, 6, 3, 7]  # head order: pair-major

_built = {}


def _build_nc():
    import concourse.mybir as mybir
    import concourse.tile as tile
    from concourse import bacc

    fp32 = mybir.dt.float32
    bf16 = mybir.dt.bfloat16
    Exp = mybir.ActivationFunctionType.Exp

    nc = bacc.Bacc("TRN2", target_bir_lowering=False, debug=False,
                   num_devices=NCORES)

    xT = nc.dram_tensor("xT", [D, S], bf16, kind="ExternalInput").ap()
    wq = nc.dram_tensor("wq", [D, D], bf16, kind="ExternalInput").ap()
    wk = nc.dram_tensor("wk", [D, KV * DH], bf16, kind="ExternalInput").ap()
    wv = nc.dram_tensor("wv", [D, KV * DH], bf16, kind="ExternalInput").ap()
    wo = nc.dram_tensor("wo", [D, D], bf16, kind="ExternalInput").ap()
    bqp = nc.dram_tensor("bqp", [128, PAIRS], fp32, kind="ExternalInput").ap()
    bkvp = nc.dram_tensor("bkvp", [128, 1], fp32, kind="ExternalInput").ap()
    bvbc = nc.dram_tensor("bvbc", [128, 128], fp32, kind="ExternalInput").ap()
    bobc = nc.dram_tensor("bobc", [128, D], fp32, kind="ExternalInput").ap()
    y = nc.dram_tensor("y", [SQ, D], fp32, kind="ExternalOutput").ap()

    with tile.TileContext(nc) as tc:
        with (
            tc.tile_pool(name="consts", bufs=1) as consts,
            tc.tile_pool(name="epool", bufs=3) as epool,
            tc.tile_pool(name="opool", bufs=6) as opool,
            tc.tile_pool(name="rpool", bufs=2) as rpool,
            tc.tile_pool(name="ypool", bufs=3) as ypool,
            tc.tile_pool(name="pssc", bufs=2, space="PSUM") as pssc,
            tc.tile_pool(name="ps512", bufs=4, space="PSUM") as ps512,
        ):
            # ---- load constants / inputs ----
            # weights + biases first (small), then x in 4 column chunks so
            # projections can start after the first ~1.5us of x DMA.
            wq_sb = consts.tile([128, 4, D], bf16, tag="wq")
            nc.sync.dma_start(wq_sb[:], wq.rearrange("(c p) j -> p c j", p=128))
            wk_sb = consts.tile([128, 4, 128], bf16, tag="wk")
            nc.sync.dma_start(wk_sb[:], wk.rearrange("(c p) j -> p c j", p=128))
            wv_sb = consts.tile([128, 4, 128], bf16, tag="wv")
            nc.sync.dma_start(wv_sb[:], wv.rearrange("(c p) j -> p c j", p=128))
            wo_sb = consts.tile([128, 4, D], bf16, tag="wo")
            nc.sync.dma_start(wo_sb[:], wo.rearrange("(c p) j -> p c j", p=128))
            bq_sb = consts.tile([128, PAIRS], fp32, tag="bq")
            nc.sync.dma_start(bq_sb[:], bqp)
            bkv_sb = consts.tile([128, 1], fp32, tag="bkv")
            nc.sync.dma_start(bkv_sb[:], bkvp)
            bv_sb = consts.tile([128, 128], fp32, tag="bv")
            nc.sync.dma_start(bv_sb[:], bvbc)
            bo_sb = consts.tile([128, D], fp32, tag="bo")
            nc.sync.dma_start(bo_sb[:], bobc)
            ones_sb = consts.tile([128, DH], bf16, tag="ones")
            nc.vector.memset(ones_sb[:], 1.0)

            xt_sb = consts.tile([128, 4, S], bf16, tag="xt")
            xT_r = xT.rearrange("(c p) s -> p c s", p=128)
            for sc in range(4):
                nc.sync.dma_start(xt_sb[:, :, sc * 512:(sc + 1) * 512],
                                  xT_r[:, :, sc * 512:(sc + 1) * 512])

            # ---- projections (interleaved with x DMA chunks) ----
            kt_sb = consts.tile([128, S], bf16, tag="kt")
            v_sb = consts.tile([128, S // 128, 128], bf16, tag="v")
            qt_sb = consts.tile([128, PAIRS, SQ], bf16, tag="qt")

            def kt_chunk(sc):
                # kT [128 (kv0|kv1 head-dim), S] columns sc*512:(sc+1)*512
                ps = ps512.tile([128, 512], fp32, tag="ps512")
                for c in range(4):
                    nc.tensor.matmul(ps[:], wk_sb[:, c, :],
                                     xt_sb[:, c, sc * 512:(sc + 1) * 512],
                                     start=(c == 0), stop=(c == 3))
                nc.vector.tensor_scalar_add(kt_sb[:, sc * 512:(sc + 1) * 512],
                                            ps[:], bkv_sb[:, 0:1])

            def v_block(sb):
                # v natural [s-block, 128][(kv0|kv1) head-dim]
                ps = ps512.tile([128, 512], fp32, tag="ps512")
                for c in range(4):
                    nc.tensor.matmul(ps[:, 0:128],
                                     xt_sb[:, c, sb * 128:(sb + 1) * 128],
                                     wv_sb[:, c, :],
                                     start=(c == 0), stop=(c == 3))
                nc.vector.tensor_add(v_sb[:, sb, :], ps[:, 0:128], bv_sb[:])

            def qt_chunk(pr, sc):
                # qT [128 (head p | head p+4), SQ] columns sc*512:(sc+1)*512
                ps = ps512.tile([128, 512], fp32, tag="ps512")
                for c in range(4):
                    nc.tensor.matmul(ps[:],
                                     wq_sb[:, c, pr * 128:(pr + 1) * 128],
                                     xt_sb[:, c, sc * 512:(sc + 1) * 512],
                                     start=(c == 0), stop=(c == 3))
                nc.vector.tensor_scalar_add(
                    qt_sb[:, pr, sc * 512:(sc + 1) * 512], ps[:],
                    bq_sb[:, pr:pr + 1])

            for sc in range(4):
                kt_chunk(sc)
                if sc < 2:
                    for pr in range(PAIRS):
                        qt_chunk(pr, sc)
                for sb in range(4 * sc, 4 * sc + 4):
                    v_block(sb)

            # ---- attention + o-proj ----
            NKB = S // 128  # 16 key blocks
            for qc in range(SQ // 512):
                ot_tiles = []
                for pr in range(PAIRS):
                    acc = ps512.tile([128, 512], fp32, tag="ps512")
                    den = ps512.tile([128, 512], fp32, tag="ps512")
                    e_tiles = [None] * NKB

                    def attnv(kb):
                        e = e_tiles[kb]
                        nc.tensor.matmul(acc[0:64, :], v_sb[:, kb, 0:64],
                                         e[:, 0:512],
                                         start=(kb == 0), stop=(kb == NKB - 1),
                                         tile_position=(0, 0))
                        nc.tensor.matmul(acc[64:128, :], v_sb[:, kb, 64:128],
                                         e[:, 512:1024],
                                         start=(kb == 0), stop=(kb == NKB - 1),
                                         tile_position=(0, 64))
                        nc.tensor.matmul(den[0:64, :], ones_sb[:],
                                         e[:, 0:512],
                                         start=(kb == 0), stop=(kb == NKB - 1),
                                         tile_position=(0, 0))
                        nc.tensor.matmul(den[64:128, :], ones_sb[:],
                                         e[:, 512:1024],
                                         start=(kb == 0), stop=(kb == NKB - 1),
                                         tile_position=(0, 64))

                    for kb in range(NKB):
                        sc_ps = pssc.tile([128, 1024], fp32, tag="scores")
                        nc.tensor.matmul(
                            sc_ps[:, 0:512],
                            kt_sb[0:64, kb * 128:(kb + 1) * 128],
                            qt_sb[0:64, pr, qc * 512:(qc + 1) * 512])
                        nc.tensor.matmul(
                            sc_ps[:, 512:1024],
                            kt_sb[64:128, kb * 128:(kb + 1) * 128],
                            qt_sb[64:128, pr, qc * 512:(qc + 1) * 512])
                        e = epool.tile([128, 1024], bf16, tag="E")
                        e_tiles[kb] = e
                        nc.scalar.activation(e[:], sc_ps[:], Exp, scale=SCALE)
                        # software pipeline: consume previous block's probs so
                        # PE never waits on the exp of the current block
                        if kb >= 1:
                            attnv(kb - 1)
                    attnv(NKB - 1)

                    rb = rpool.tile([128, 512], fp32, tag="recip")
                    scr = rpool.tile([128, 512], fp32, tag="rscr")
                    nc.vector.reciprocal_approx_accurate(rb[:], den[:], scr[:])
                    ot = opool.tile([128, 512], bf16, tag="outT")
                    nc.vector.tensor_mul(ot[:], acc[:], rb[:])
                    ot_tiles.append(ot)
                for m in range(4):
                    yp = ps512.tile([128, 512], fp32, tag="ps512")
                    for pr2 in range(PAIRS):
                        nc.tensor.matmul(yp[:],
                                         ot_tiles[pr2][:, m * 128:(m + 1) * 128],
                                         wo_sb[:, pr2, :],
                                         start=(pr2 == 0), stop=(pr2 == 3))
                    yt = ypool.tile([128, 512], fp32, tag="y")
                    nc.vector.tensor_add(yt[:], yp[:], bo_sb[:])
                    blk = qc * 4 + m
                    nc.sync.dma_start(y[blk * 128:(blk + 1) * 128, :], yt[:])

    nc.finalize()
    return nc


def _get_nc():
    if "nc" not in _built:
        _built["nc"] = _build_nc()
    return _built["nc"]


def _prep_in_maps(x, Wq, bq, Wk, bk, Wv, bv, Wo, bo):
    import ml_dtypes

    bf16 = ml_dtypes.bfloat16

    x = np.ascontiguousarray(np.asarray(x, np.float32))
    Wq = np.asarray(Wq, np.float32)
    bq = np.asarray(bq, np.float32)
    Wk = np.asarray(Wk, np.float32)
    bk = np.asarray(bk, np.float32)
    Wv = np.asarray(Wv, np.float32)
    bv = np.asarray(bv, np.float32)
    Wo = np.asarray(Wo, np.float32)
    bo = np.asarray(bo, np.float32)

    wq_p = np.ascontiguousarray(
        Wq.reshape(D, H, DH)[:, PERM, :].reshape(D, D).astype(bf16))
    wo_p = np.ascontiguousarray(
        Wo.reshape(H, DH, D)[PERM].reshape(D, D).astype(bf16))
    wk_b = np.ascontiguousarray(Wk.astype(bf16))
    wv_b = np.ascontiguousarray(Wv.astype(bf16))
    bq_p = np.ascontiguousarray(
        bq.reshape(H, DH)[PERM].reshape(PAIRS, 128).T)
    bkv_p = np.ascontiguousarray(bk.reshape(128, 1))
    bv_bc = np.ascontiguousarray(np.tile(bv[None, :], (128, 1)))
    bo_bc = np.ascontiguousarray(np.tile(bo[None, :], (128, 1)))

    in_maps = []
    for c in range(NCORES):
        b, sh = divmod(c, 2)
        xroll = np.roll(x[b], -sh * SQ, axis=0)
        in_maps.append({
            "xT": np.ascontiguousarray(xroll.T.astype(bf16)),
            "wq": wq_p, "wk": wk_b, "wv": wv_b, "wo": wo_p,
            "bqp": bq_p, "bkvp": bkv_p, "bvbc": bv_bc, "bobc": bo_bc,
        })
    return in_maps


def kernel(x, Wq, bq, Wk, bk, Wv, bv, Wo, bo):
    from concourse.bass_utils import run_bass_kernel_spmd

    in_maps = _prep_in_maps(x, Wq, bq, Wk, bk, Wv, bv, Wo, bo)
    nc = _get_nc()
    res = run_bass_kernel_spmd(nc, in_maps, list(range(NCORES)))
    out = np.empty((B, S, D), np.float32)
    for c in range(NCORES):
        b, sh = divmod(c, 2)
        out[b, sh * SQ:(sh + 1) * SQ, :] = res.results[c]["y"]
    return out


# revision 5
# speedup vs baseline: 2.5949x; 1.0140x over previous
"""GroupedQueryAttention kernel for 8 Trainium2 NeuronCores.

Sharding: core c = (batch b = c//2, seq-half sh = c%2). Each core computes the
full attention output for 1024 query rows of one batch: all 8 q heads
(2 kv heads), plus the q/k/v projections and the o-projection for those rows.
Host work is limited to slicing/transposing/casting inputs and concatenating
outputs.

All matmul operands are bf16 (fp32 matmuls run as two PE passes on TRN2 —
bf16 halves tensor-engine time); accumulation stays fp32 in PSUM and the
softmax denominator/reciprocal stays fp32.

Inputs stream in chunks (weights needed first come first, x in 4 column
chunks held in separate tiles) so the projection matmuls start ~8us in
instead of waiting for the whole 2MB x DMA; the attention inner loop is
paced by the scalar engine's exp throughput (1 elem/cycle/lane), with the
tensor engine ~90% subscribed underneath it.

On-device layout: scoresT [keys, queries] so softmax-exp'd probabilities feed
attn@v matmuls directly as the moving operand (no transposes anywhere).
Denominators are produced by ones-weight matmuls replicated across all 64
output partitions, so normalization is a plain elementwise multiply.
"""

import numpy as np

B, S, D = 4, 2048, 512
H, KV, DH = 8, 2, 64
SQ = S // 2  # queries per core
NCORES = 8
PAIRS = 4  # head pairs (p, p+4); p -> kv0 rows 0:64, p+4 -> kv1 rows 64:128
SCALE = 1.0 / 8.0  # 1/sqrt(DH)
PERM = [0, 4, 1, 5, 2, 6, 3, 7]  # head order: pair-major
NKB = S // 128  # 16 key blocks

_built = {}


def _build_nc():
    import concourse.mybir as mybir
    import concourse.tile as tile
    from concourse import bacc

    fp32 = mybir.dt.float32
    bf16 = mybir.dt.bfloat16
    Exp = mybir.ActivationFunctionType.Exp

    nc = bacc.Bacc("TRN2", target_bir_lowering=False, debug=False,
                   num_devices=NCORES)

    xT = nc.dram_tensor("xT", [D, S], bf16, kind="ExternalInput").ap()
    wq = nc.dram_tensor("wq", [D, D], bf16, kind="ExternalInput").ap()
    wk = nc.dram_tensor("wk", [D, KV * DH], bf16, kind="ExternalInput").ap()
    wv = nc.dram_tensor("wv", [D, KV * DH], bf16, kind="ExternalInput").ap()
    wo = nc.dram_tensor("wo", [D, D], bf16, kind="ExternalInput").ap()
    bqp = nc.dram_tensor("bqp", [128, PAIRS], fp32, kind="ExternalInput").ap()
    bkvp = nc.dram_tensor("bkvp", [128, 1], fp32, kind="ExternalInput").ap()
    bvbc = nc.dram_tensor("bvbc", [128, 128], fp32, kind="ExternalInput").ap()
    bobc = nc.dram_tensor("bobc", [128, D], fp32, kind="ExternalInput").ap()
    y = nc.dram_tensor("y", [SQ, D], bf16, kind="ExternalOutput").ap()

    with tile.TileContext(nc) as tc:
        with (
            tc.tile_pool(name="consts", bufs=1) as consts,
            tc.tile_pool(name="epool", bufs=3) as epool,
            tc.tile_pool(name="opool", bufs=8) as opool,
            tc.tile_pool(name="rpool", bufs=2) as rpool,
            tc.tile_pool(name="ypool", bufs=3) as ypool,
            tc.tile_pool(name="pssc", bufs=2, space="PSUM") as pssc,
            tc.tile_pool(name="ps512", bufs=4, space="PSUM") as ps512,
        ):
            # ---- input DMAs, ordered by first use ----
            xT_r = xT.rearrange("(c p) s -> p c s", p=128)
            xt_c = [consts.tile([128, 4, 512], bf16, name=f"xt{i}", tag=f"xt{i}")
                    for i in range(4)]

            wk_sb = consts.tile([128, 4, 128], bf16, tag="wk")
            nc.sync.dma_start(wk_sb[:], wk.rearrange("(c p) j -> p c j", p=128))
            bkv_sb = consts.tile([128, 1], fp32, tag="bkv")
            nc.sync.dma_start(bkv_sb[:], bkvp)
            nc.sync.dma_start(xt_c[0][:], xT_r[:, :, 0:512])
            wq_sb = consts.tile([128, 4, D], bf16, tag="wq")
            nc.sync.dma_start(wq_sb[:], wq.rearrange("(c p) j -> p c j", p=128))
            bq_sb = consts.tile([128, PAIRS], fp32, tag="bq")
            nc.sync.dma_start(bq_sb[:], bqp)
            nc.sync.dma_start(xt_c[1][:], xT_r[:, :, 512:1024])
            wv_sb = consts.tile([128, 4, 128], bf16, tag="wv")
            nc.sync.dma_start(wv_sb[:], wv.rearrange("(c p) j -> p c j", p=128))
            bv_sb = consts.tile([128, 128], fp32, tag="bv")
            nc.sync.dma_start(bv_sb[:], bvbc)
            nc.sync.dma_start(xt_c[2][:], xT_r[:, :, 1024:1536])
            nc.sync.dma_start(xt_c[3][:], xT_r[:, :, 1536:2048])
            wo_sb = consts.tile([128, 4, D], bf16, tag="wo")
            nc.sync.dma_start(wo_sb[:], wo.rearrange("(c p) j -> p c j", p=128))
            bo_sb = consts.tile([128, D], fp32, tag="bo")
            nc.sync.dma_start(bo_sb[:], bobc)
            ones_sb = consts.tile([128, DH], bf16, tag="ones")
            nc.vector.memset(ones_sb[:], 1.0)

            # ---- projections (per x column-chunk, separate dest tiles) ----
            kt_c = [consts.tile([128, 512], bf16, name=f"kt{i}", tag=f"kt{i}")
                    for i in range(4)]
            v_c = [consts.tile([128, 4, 128], bf16, name=f"v{i}", tag=f"v{i}")
                   for i in range(4)]
            qt_c = {}

            def kt_chunk(sc):
                # kT [128 (kv0|kv1 head-dim), 512 keys]
                ps = ps512.tile([128, 512], fp32, tag="ps512")
                for c in range(4):
                    nc.tensor.matmul(ps[:], wk_sb[:, c, :], xt_c[sc][:, c, :],
                                     start=(c == 0), stop=(c == 3))
                nc.vector.tensor_scalar_add(kt_c[sc][:], ps[:], bkv_sb[:, 0:1])

            def v_chunk(sc):
                # v natural [s-block, 128][(kv0|kv1) head-dim], 4 blocks
                for sb in range(4):
                    ps = ps512.tile([128, 512], fp32, tag="ps512")
                    for c in range(4):
                        nc.tensor.matmul(
                            ps[:, 0:128],
                            xt_c[sc][:, c, sb * 128:(sb + 1) * 128],
                            wv_sb[:, c, :],
                            start=(c == 0), stop=(c == 3))
                    nc.vector.tensor_add(v_c[sc][:, sb, :], ps[:, 0:128],
                                         bv_sb[:])

            def qt_chunk(pr, sc):
                # qT [128 (head p | head p+4), 512 queries]
                t = qt_c[(pr, sc)] = consts.tile([128, 512], bf16,
                                                 name=f"qt{pr}_{sc}",
                                                 tag=f"qt{pr}_{sc}")
                ps = ps512.tile([128, 512], fp32, tag="ps512")
                for c in range(4):
                    nc.tensor.matmul(ps[:],
                                     wq_sb[:, c, pr * 128:(pr + 1) * 128],
                                     xt_c[sc][:, c, :],
                                     start=(c == 0), stop=(c == 3))
                nc.vector.tensor_scalar_add(t[:], ps[:], bq_sb[:, pr:pr + 1])

            kt_chunk(0)
            for pr in range(PAIRS):
                qt_chunk(pr, 0)
            v_chunk(0)
            kt_chunk(1)
            for pr in range(PAIRS):
                qt_chunk(pr, 1)
            v_chunk(1)
            kt_chunk(2)
            v_chunk(2)
            kt_chunk(3)
            v_chunk(3)

            # ---- attention (qc = query 512-chunk, pr = head pair) ----
            def attention_pair(qc, pr):
                acc = ps512.tile([128, 512], fp32, tag="ps512")
                den = ps512.tile([128, 512], fp32, tag="ps512")
                e_tiles = [None] * NKB
                qt0 = qt_c[(pr, qc)]

                def attnv(kb):
                    e = e_tiles[kb]
                    vt = v_c[kb // 4]
                    sb = kb % 4
                    nc.tensor.matmul(acc[0:64, :], vt[:, sb, 0:64],
                                     e[:, 0:512],
                                     start=(kb == 0), stop=(kb == NKB - 1),
                                     tile_position=(0, 0))
                    nc.tensor.matmul(acc[64:128, :], vt[:, sb, 64:128],
                                     e[:, 512:1024],
                                     start=(kb == 0), stop=(kb == NKB - 1),
                                     tile_position=(0, 64))
                    nc.tensor.matmul(den[0:64, :], ones_sb[:],
                                     e[:, 0:512],
                                     start=(kb == 0), stop=(kb == NKB - 1),
                                     tile_position=(0, 0))
                    nc.tensor.matmul(den[64:128, :], ones_sb[:],
                                     e[:, 512:1024],
                                     start=(kb == 0), stop=(kb == NKB - 1),
                                     tile_position=(0, 64))

                for kb in range(NKB):
                    kt = kt_c[kb // 4]
                    kcol = (kb % 4) * 128
                    sc_ps = pssc.tile([128, 1024], fp32, tag="scores")
                    nc.tensor.matmul(sc_ps[:, 0:512],
                                     kt[0:64, kcol:kcol + 128], qt0[0:64, :])
                    nc.tensor.matmul(sc_ps[:, 512:1024],
                                     kt[64:128, kcol:kcol + 128],
                                     qt0[64:128, :])
                    e = epool.tile([128, 1024], bf16, tag="E")
                    e_tiles[kb] = e
                    nc.scalar.activation(e[:], sc_ps[:], Exp, scale=SCALE)
                    # software pipeline: consume previous block's probs so
                    # PE never waits on the exp of the current block
                    if kb >= 1:
                        attnv(kb - 1)
                attnv(NKB - 1)

                rb = rpool.tile([128, 512], fp32, tag="recip")
                scr = rpool.tile([128, 512], fp32, tag="rscr")
                nc.vector.reciprocal_approx_accurate(rb[:], den[:], scr[:])
                ot = opool.tile([128, 512], bf16, tag="outT")
                nc.vector.tensor_mul(ot[:], acc[:], rb[:])
                return ot

            def o_proj_group(ots, qc, m):
                yp = ps512.tile([128, 512], fp32, tag="ps512")
                for pr2 in range(PAIRS):
                    nc.tensor.matmul(yp[:], ots[pr2][:, m * 128:(m + 1) * 128],
                                     wo_sb[:, pr2, :],
                                     start=(pr2 == 0), stop=(pr2 == 3))
                yt = ypool.tile([128, 512], bf16, tag="y")
                nc.vector.tensor_add(yt[:], yp[:], bo_sb[:])
                blk = qc * 4 + m
                nc.sync.dma_start(y[blk * 128:(blk + 1) * 128, :], yt[:])

            ots0 = [attention_pair(0, pr) for pr in range(PAIRS)]
            ots1 = []
            for pr in range(PAIRS):
                ots1.append(attention_pair(1, pr))
                # qc0's o-projection rides in qc1's tensor-engine slack so the
                # scalar engine never stalls at the qc boundary
                o_proj_group(ots0, 0, pr)
            for m in range(4):
                o_proj_group(ots1, 1, m)

    nc.finalize()
    return nc


def _get_nc():
    if "nc" not in _built:
        _built["nc"] = _build_nc()
    return _built["nc"]


def _prep_in_maps(x, Wq, bq, Wk, bk, Wv, bv, Wo, bo):
    import ml_dtypes

    bf16 = ml_dtypes.bfloat16

    x = np.ascontiguousarray(np.asarray(x, np.float32))
    Wq = np.asarray(Wq, np.float32)
    bq = np.asarray(bq, np.float32)
    Wk = np.asarray(Wk, np.float32)
    bk = np.asarray(bk, np.float32)
    Wv = np.asarray(Wv, np.float32)
    bv = np.asarray(bv, np.float32)
    Wo = np.asarray(Wo, np.float32)
    bo = np.asarray(bo, np.float32)

    wq_p = np.ascontiguousarray(
        Wq.reshape(D, H, DH)[:, PERM, :].reshape(D, D).astype(bf16))
    wo_p = np.ascontiguousarray(
        Wo.reshape(H, DH, D)[PERM].reshape(D, D).astype(bf16))
    wk_b = np.ascontiguousarray(Wk.astype(bf16))
    wv_b = np.ascontiguousarray(Wv.astype(bf16))
    bq_p = np.ascontiguousarray(
        bq.reshape(H, DH)[PERM].reshape(PAIRS, 128).T)
    bkv_p = np.ascontiguousarray(bk.reshape(128, 1))
    bv_bc = np.ascontiguousarray(np.tile(bv[None, :], (128, 1)))
    bo_bc = np.ascontiguousarray(np.tile(bo[None, :], (128, 1)))

    in_maps = []
    for c in range(NCORES):
        b, sh = divmod(c, 2)
        xroll = np.roll(x[b], -sh * SQ, axis=0)
        in_maps.append({
            "xT": np.ascontiguousarray(xroll.T.astype(bf16)),
            "wq": wq_p, "wk": wk_b, "wv": wv_b, "wo": wo_p,
            "bqp": bq_p, "bkvp": bkv_p, "bvbc": bv_bc, "bobc": bo_bc,
        })
    return in_maps


def kernel(x, Wq, bq, Wk, bk, Wv, bv, Wo, bo):
    from concourse.bass_utils import run_bass_kernel_spmd

    in_maps = _prep_in_maps(x, Wq, bq, Wk, bk, Wv, bv, Wo, bo)
    nc = _get_nc()
    res = run_bass_kernel_spmd(nc, in_maps, list(range(NCORES)))
    out = np.empty((B, S, D), np.float32)
    for c in range(NCORES):
        b, sh = divmod(c, 2)
        out[b, sh * SQ:(sh + 1) * SQ, :] = \
            np.asarray(res.results[c]["y"]).astype(np.float32)
    return out
